# revision 1
# baseline (speedup 1.0000x reference)
"""Llama3 attention prefill kernel for 8 Trainium2 NeuronCores.

Sharding: tensor-parallel over heads. Core c owns Q heads 4c..4c+3 and KV
head c (GQA group), plus the matching wqkv columns / wo rows. Each core
computes a partial output y_c = attn_c @ wo_c; the host sums the partials.

Per-core pipeline (all inside one TileContext):
  1. qkv = x @ w_shard   (fp16 matmuls, xT tiles produced by PE transpose)
  2. RoPE on q/k in [S, head*64*2] layout (DVE), transpose q/k to [HD, S]
  3. causal flash attention per (head, q-tile): scores -> exp (+row sums)
     -> normalize -> PE-transpose P -> P^T @ v accumulation (out^T layout)
  4. y^T = wo_shard^T @ out^T  (float32r matmuls), DMA y^T back
"""

import os
import sys

for _p in ("/opt/trn_rl_repo", "/root/.axon_site/_ro/trn_rl_repo"):
    if os.path.isdir(_p) and _p not in sys.path:
        sys.path.insert(0, _p)

import numpy as np

S = 2048
H = 4096
HD = 128
NQ = 4            # q heads per core
MQKV = 768        # per-core qkv columns: 512 q + 128 k + 128 v
N_CORES = 8
SCALE = 1.0 / float(np.sqrt(HD))
MASK_VAL = -1e9

_CACHE = {}
LAST_RESULTS = None


def _build():
    import concourse.tile as tile
    from concourse import bacc, mybir
    from concourse.masks import make_causal_mask, make_identity

    f32 = mybir.dt.float32
    f16 = mybir.dt.float16
    Exp = mybir.ActivationFunctionType.Exp

    nc = bacc.Bacc("TRN2", target_bir_lowering=False, debug=False)

    x_ap = nc.dram_tensor("x", [S, H], f32, kind="ExternalInput").ap()
    w_ap = nc.dram_tensor("w", [H, MQKV], f32, kind="ExternalInput").ap()
    wo_ap = nc.dram_tensor("wo", [NQ * HD, H], f32, kind="ExternalInput").ap()
    cs_ap = nc.dram_tensor("cs5", [S, 320], f32, kind="ExternalInput").ap()
    sn_ap = nc.dram_tensor("sn5", [S, 320], f32, kind="ExternalInput").ap()
    yT_ap = nc.dram_tensor("yT", [H, S], f32, kind="ExternalOutput").ap()

    KT = S // 128    # 16 m-tiles over S
    KC = H // 128    # 32 contraction chunks for qkv

    with tile.TileContext(nc) as tc:
        from contextlib import ExitStack

        with ExitStack() as ctx:
            const = ctx.enter_context(tc.tile_pool(name="const", bufs=1))
            ident = const.tile([128, 128], f16)
            make_identity(nc, ident[:])
            cmask = const.tile([128, 128], f32)
            make_causal_mask(nc, cmask[:], mask_val=MASK_VAL)

            # resident tensors
            res = ctx.enter_context(tc.tile_pool(name="res", bufs=1))
            qkT_sb = res.tile([128, 5, KT, 128], f16, name="qkT_sb")
            v_sb = res.tile([128, KT, 128], f16, name="v_sb")
            outT_sb = res.tile([128, NQ, S], f16, name="outT_sb")
            w_pool_cm = tc.tile_pool(name="w_pool", bufs=1, side="right")
            w_pool = w_pool_cm.__enter__()
            w_sb = w_pool.tile([128, KC, MQKV], f16, name="w_sb")

            # streaming pools
            wst = ctx.enter_context(tc.tile_pool(name="wst", bufs=3))
            qsbp = ctx.enter_context(tc.tile_pool(name="qsbp", bufs=2))
            xp = ctx.enter_context(tc.tile_pool(name="xp", bufs=6))
            xfp = ctx.enter_context(tc.tile_pool(name="xfp", bufs=10))
            xtp = ctx.enter_context(tc.tile_pool(name="xtp", bufs=12))
            csp = ctx.enter_context(tc.tile_pool(name="csp", bufs=2))
            rtp = ctx.enter_context(tc.tile_pool(name="rtp", bufs=2))
            rotp = ctx.enter_context(tc.tile_pool(name="rotp", bufs=2))
            pp = ctx.enter_context(tc.tile_pool(name="pp", bufs=5))
            ptp = ctx.enter_context(tc.tile_pool(name="ptp", bufs=3))
            lp = ctx.enter_context(tc.tile_pool(name="lp", bufs=3))
            yp = ctx.enter_context(tc.tile_pool(name="yp", bufs=3))

            # PSUM pools: 4 + 2 + 1 + 1 = 8 banks
            ps_big = ctx.enter_context(
                tc.tile_pool(name="ps_big", bufs=4, space="PSUM"))
            ps_qkv = ctx.enter_context(
                tc.tile_pool(name="ps_qkv", bufs=1, space="PSUM"))
            ps_tr = ctx.enter_context(
                tc.tile_pool(name="ps_tr", bufs=1, space="PSUM"))
            ps_pv = ctx.enter_context(
                tc.tile_pool(name="ps_pv", bufs=1, space="PSUM"))

            # ---- phase 1: qkv projection + rope + q/k transpose ----
            # software-pipelined: x for tile t+1 is loaded/cast/transposed
            # while tile t's matmuls run, so PE never waits at tile bounds.
            def x_load(t):
                casts = []
                for c8 in range(KC // 4):
                    xt = xp.tile([128, 512], f32, tag="xt", name=f"xt{t}_{c8}")
                    nc.sync.dma_start(
                        out=xt[:],
                        in_=x_ap[t * 128:(t + 1) * 128, c8 * 512:(c8 + 1) * 512])
                    if t == 0:
                        for kcw in range(c8 * 4, c8 * 4 + 4):
                            wt = wst.tile([128, MQKV], f32, tag="wt",
                                          name=f"wt{kcw}")
                            nc.sync.dma_start(
                                out=wt[:],
                                in_=w_ap[kcw * 128:(kcw + 1) * 128, :])
                            nc.vector.tensor_copy(out=w_sb[:, kcw, :],
                                                  in_=wt[:])
                    xf = xfp.tile([128, 512], f16, tag="xf", name=f"xf{t}_{c8}")
                    nc.vector.tensor_copy(out=xf[:], in_=xt[:])
                    casts.append(xf)
                return casts

            def x_transpose(t, casts):
                outs = []
                for c8 in range(KC // 4):
                    xf = casts[c8]
                    tr = ps_tr.tile([128, 512], f16, tag="tr",
                                    name=f"xtr{t}_{c8}")
                    for c4 in range(4):
                        nc.tensor.transpose(
                            tr[:, c4 * 128:(c4 + 1) * 128],
                            xf[:, c4 * 128:(c4 + 1) * 128], ident[:])
                    xT = xtp.tile([128, 512], f16, tag="xT",
                                  name=f"xT{t}_{c8}")
                    nc.vector.tensor_copy(out=xT[:], in_=tr[:])
                    outs.append(xT)
                return outs

            xT_cur = x_transpose(0, x_load(0))
            for t in range(KT):
                if t + 1 < KT:
                    casts_next = x_load(t + 1)
                qkv_ps = ps_qkv.tile([128, MQKV], f32, tag="qkv")
                for kc in range(KC):
                    lhsT = xT_cur[kc // 4][:, (kc % 4) * 128:(kc % 4 + 1) * 128]
                    nc.tensor.matmul(
                        qkv_ps[:, 0:512], lhsT=lhsT, rhs=w_sb[:, kc, 0:512],
                        start=(kc == 0), stop=(kc == KC - 1))
                    nc.tensor.matmul(
                        qkv_ps[:, 512:768], lhsT=lhsT, rhs=w_sb[:, kc, 512:768],
                        start=(kc == 0), stop=(kc == KC - 1))
                if t + 1 < KT:
                    xT_cur = x_transpose(t + 1, casts_next)

                # evict full qkv psum to SBUF fast (frees PSUM for next tile)
                qkv_sb = qsbp.tile([128, MQKV], f32, tag="qkv_sb")
                nc.scalar.copy(out=qkv_sb[:], in_=qkv_ps[:])
                # v eviction (no rope)
                nc.scalar.copy(out=v_sb[:, t, :], in_=qkv_sb[:, 640:768])

                # rope on q (4 heads) + k (1 head), pairs interleaved along free
                cs_t = csp.tile([128, 320], f32, tag="cs")
                nc.sync.dma_start(out=cs_t[:], in_=cs_ap[t * 128:(t + 1) * 128, :])
                sn_t = csp.tile([128, 320], f32, tag="sn")
                nc.sync.dma_start(out=sn_t[:], in_=sn_ap[t * 128:(t + 1) * 128, :])

                qk = qkv_sb[:, 0:640].rearrange("p (n two) -> p n two", two=2)
                qe = qk[:, :, 0]
                qo = qk[:, :, 1]
                rot = rotp.tile([128, 640], f16, tag="rot")
                rv = rot[:].rearrange("p (n two) -> p n two", two=2)
                t1 = rtp.tile([128, 320], f32, tag="t1")
                t2 = rtp.tile([128, 320], f32, tag="t2")
                nc.vector.tensor_mul(t1[:], qe, cs_t[:])
                nc.vector.tensor_mul(t2[:], qo, sn_t[:])
                nc.vector.scalar_tensor_tensor(
                    rv[:, :, 0], t2[:], -1.0, t1[:],
                    op0=mybir.AluOpType.mult, op1=mybir.AluOpType.add)
                nc.vector.tensor_mul(t1[:], qo, cs_t[:])
                nc.vector.tensor_mul(t2[:], qe, sn_t[:])
                nc.vector.tensor_add(rv[:, :, 1], t1[:], t2[:])

                # transpose rope'd q/k into [HD, head, t, 128] resident layout
                tr2 = ps_tr.tile([128, 512], f16, tag="tr")
                for h in range(4):
                    nc.tensor.transpose(
                        tr2[:, h * 128:(h + 1) * 128],
                        rot[:, h * 128:(h + 1) * 128], ident[:])
                nc.vector.tensor_copy(
                    out=qkT_sb[:, 0:4, t, :],
                    in_=tr2[:].rearrange("p (h s) -> p h s", h=4))
                tr3 = ps_tr.tile([128, 512], f16, tag="tr")
                nc.tensor.transpose(tr3[:, 0:128], rot[:, 512:640], ident[:])
                nc.vector.tensor_copy(out=qkT_sb[:, 4, t, :], in_=tr3[:, 0:128])

            # ---- phase 1 done: release w_sb space, load wo shard there
            w_pool_cm.__exit__(None, None, None)
            wo_pool = ctx.enter_context(tc.tile_pool(name="wo_pool", bufs=1, side="right"))
            wo_sb = wo_pool.tile([128, NQ, H], f16, name="wo_sb")
            wol = ctx.enter_context(tc.tile_pool(name="wol", bufs=2))
            for kc in range(NQ):
                for hh in range(4):
                    wt3 = wol.tile([128, 1024], f32, tag="wt3")
                    nc.sync.dma_start(
                        out=wt3[:],
                        in_=wo_ap[kc * 128:(kc + 1) * 128,
                                  hh * 1024:(hh + 1) * 1024])
                    nc.vector.tensor_copy(
                        out=wo_sb[:, kc, hh * 1024:(hh + 1) * 1024], in_=wt3[:])

            # ---- phase 2+3: causal flash attention with interleaved output
            # projection (y chunk q4 emitted once q-tiles 4*q4..4*q4+3 done)
            kT_flat = qkT_sb[:, 4, :, :].rearrange("p a b -> p (a b)")
            for i in range(KT):
                L = (i + 1) * 128
                nch = (L + 511) // 512
                Ps = []
                # wave 1: scores + exp + normalization chain for all heads
                for h in range(NQ):
                    P = pp.tile([128, S], f16, tag="P", name=f"P{i}_{h}")
                    lacc = lp.tile([128, 4], f32, tag="l")
                    for ch in range(nch):
                        c0 = ch * 512
                        c1 = min(L, c0 + 512)
                        sps = ps_big.tile([128, 512], f32, tag="big")
                        nc.tensor.matmul(
                            sps[:, 0:c1 - c0],
                            lhsT=qkT_sb[:, h, i, :],
                            rhs=kT_flat[:, c0:c1],
                            start=True, stop=True)
                        if c1 == L:
                            # diagonal block: additive causal mask
                            nc.vector.tensor_add(
                                sps[:, L - 128 - c0:L - c0],
                                sps[:, L - 128 - c0:L - c0], cmask[:])
                        nc.scalar.activation(
                            P[:, c0:c1], sps[:, 0:c1 - c0], Exp,
                            scale=SCALE,
                            accum_out=lacc[:, ch:ch + 1])
                    lsum = lp.tile([128, 1], f32, tag="ls")
                    if nch > 1:
                        nc.vector.tensor_reduce(
                            lsum[:], lacc[:, 0:nch],
                            axis=mybir.AxisListType.X, op=mybir.AluOpType.add)
                    else:
                        nc.vector.tensor_copy(out=lsum[:], in_=lacc[:, 0:1])
                    rinv = lp.tile([128, 1], f32, tag="r")
                    nc.vector.reciprocal(rinv[:], lsum[:])
                    nc.vector.tensor_scalar_mul(P[:, 0:L], P[:, 0:L], rinv[:])
                    Ps.append(P)
                # wave 2: transpose P and accumulate P^T @ v per head
                for h in range(NQ):
                    Pn = Ps[h]
                    PT = ptp.tile([128, S], f16, tag="PT", name=f"PT{i}_{h}")
                    for j4 in range(0, i + 1, 4):
                        jn = min(i + 1, j4 + 4)
                        trp = ps_tr.tile([128, 512], f16, tag="tr")
                        for jj in range(j4, jn):
                            nc.tensor.transpose(
                                trp[:, (jj - j4) * 128:(jj - j4 + 1) * 128],
                                Pn[:, jj * 128:(jj + 1) * 128], ident[:])
                        if (j4 // 4) % 2 == 0:
                            nc.scalar.copy(
                                out=PT[:, j4 * 128:jn * 128],
                                in_=trp[:, 0:(jn - j4) * 128])
                        else:
                            nc.vector.tensor_copy(
                                out=PT[:, j4 * 128:jn * 128],
                                in_=trp[:, 0:(jn - j4) * 128])

                    ov = ps_pv.tile([128, 128], f32, tag="pv")
                    for j in range(i + 1):
                        nc.tensor.matmul(
                            ov[:], lhsT=v_sb[:, j, :],
                            rhs=PT[:, j * 128:(j + 1) * 128],
                            start=(j == 0), stop=(j == i))
                    nc.vector.tensor_copy(
                        out=outT_sb[:, h, i * 128:(i + 1) * 128], in_=ov[:])

                yq4s = []
                if i % 4 == 3 and i >= 7:
                    yq4s = [i // 4 - 1]
                if i == KT - 1:
                    yq4s = [KT // 4 - 2, KT // 4 - 1]
                for q4 in yq4s:
                    for ym in range(H // 128):
                        yps = ps_big.tile([128, 512], f32, tag="big")
                        for kc in range(NQ):
                            nc.tensor.matmul(
                                yps[:],
                                lhsT=wo_sb[:, kc, ym * 128:(ym + 1) * 128],
                                rhs=outT_sb[:, kc, q4 * 512:(q4 + 1) * 512],
                                start=(kc == 0), stop=(kc == NQ - 1))
                        yev = yp.tile([128, 512], f32, tag="yev")
                        nc.vector.tensor_copy(out=yev[:], in_=yps[:])
                        nc.sync.dma_start(
                            out=yT_ap[ym * 128:(ym + 1) * 128,
                                      q4 * 512:(q4 + 1) * 512],
                            in_=yev[:])

    nc.compile()
    return nc


def _get_nc():
    if "nc" not in _CACHE:
        _CACHE["nc"] = _build()
    return _CACHE["nc"]


def kernel(x, last_pos, mask, rope_cache, wqkv, wo):
    global LAST_RESULTS
    from concourse.bass_utils import run_bass_kernel_spmd

    nc = _get_nc()

    x2 = np.ascontiguousarray(np.asarray(x, np.float32).reshape(S, H))
    rc = np.asarray(rope_cache, np.float32)          # [S, 64, 2]
    cos = rc[:, :, 0]                                # [S, 64]
    sin = rc[:, :, 1]
    # per-pair factors, tiled for 5 rope'd heads (4 q + 1 k): [S, 320]
    cs5 = np.ascontiguousarray(np.tile(cos, (1, 5)))
    sn5 = np.ascontiguousarray(np.tile(sin, (1, 5)))
    wq = np.asarray(wqkv, np.float32)
    wo_f = np.asarray(wo, np.float32)

    in_maps = []
    for c in range(N_CORES):
        wcat = np.concatenate(
            [wq[:, c * 512:(c + 1) * 512],
             wq[:, H + c * 128:H + (c + 1) * 128],
             wq[:, H + 1024 + c * 128:H + 1024 + (c + 1) * 128]],
            axis=1)
        in_maps.append({
            "x": x2,
            "w": np.ascontiguousarray(wcat),
            "wo": np.ascontiguousarray(wo_f[c * 512:(c + 1) * 512, :]),
            "cs5": cs5,
            "sn5": sn5,
        })

    res = run_bass_kernel_spmd(nc, in_maps, list(range(N_CORES)))
    LAST_RESULTS = res
    if res.exec_time_ns is not None:
        print(f"HW exec time: {res.exec_time_ns} ns")
    yT = res.results[0]["yT"].astype(np.float64)
    for c in range(1, N_CORES):
        yT = yT + res.results[c]["yT"]
    return np.ascontiguousarray(yT.T).reshape(1, S, H).astype(np.float32)



# revision 24
# speedup vs baseline: 1.2000x; 1.2000x over previous
"""Llama3 attention prefill kernel for 8 Trainium2 NeuronCores — v2.

Sharding: tensor-parallel over heads. Core c owns Q heads 4c..4c+3 and KV
head c (GQA group), plus the matching wqkv columns / wo rows. Each core
computes a partial output y_c = attn_c @ wo_c; the host sums the partials.

v2 design (driven by the TimelineSim cost model):
  * PE sequencer cost (~167ns/matmul) dominated v1 (3376 PE instrs), so v2
    issues ~1800 larger matmuls instead.
  * Host supplies x^T in f16, so the QKV projection runs in transposed
    layout (out = w_chunk^T @ x^T = qkv^T) with zero x-transposes and
    produces Q^T/K^T/V^T directly.
  * RoPE runs on the transposed q/k chunks: the pair-swap is one PE
    permutation matmul per chunk (host-provided swap matrix), then 3 DVE
    elementwise ops with host-precomputed interleaved cos / +-sin rows.
  * Attention computes scores TRANSPOSED (S^T[k,q] = K Q^T) per q-group of
    512 so exp (Act) writes P^T straight to SBUF — no P transposes, no
    PSUM->SBUF P evictions. Softmax denominators come from ones-vector
    matmuls accumulated in PSUM; normalization happens after PV on the
    [128,512] O^T tile via a broadcast outer-product matmul.
  * All weights/activations f16 on the wire (host pre-casts), f32 PSUM
    accumulation everywhere.
"""

import os
import sys

for _p in ("/opt/trn_rl_repo", "/root/.axon_site/_ro/trn_rl_repo"):
    if os.path.isdir(_p) and _p not in sys.path:
        sys.path.insert(0, _p)

import numpy as np

S = 2048
H = 4096
HD = 128
NQ = 4            # q heads per core
MQKV = 768        # per-core qkv columns: 512 q + 128 k + 128 v
N_CORES = 8
SCALE = 1.0 / float(np.sqrt(HD))
KC = H // 128     # 32 contraction chunks for qkv
NG = 4            # q-groups of 512
KT = S // 128     # 16 k-tiles

_CACHE = {}
LAST_RESULTS = None


def _build():
    import concourse.tile as tile
    from concourse import bacc, mybir
    from concourse.masks import make_identity

    f32 = mybir.dt.float32
    f16 = mybir.dt.float16
    Exp = mybir.ActivationFunctionType.Exp

    nc = bacc.Bacc("TRN2", target_bir_lowering=False, debug=False)

    xT_ap = nc.dram_tensor("xT", [H, S], f16, kind="ExternalInput").ap()
    w_ap = nc.dram_tensor("w", [H, MQKV], f16, kind="ExternalInput").ap()
    wo_ap = nc.dram_tensor("wo", [NQ * HD, H], f16, kind="ExternalInput").ap()
    cosI_ap = nc.dram_tensor("cosI", [128, S], f16, kind="ExternalInput").ap()
    sinI_ap = nc.dram_tensor("sinI", [128, S], f16, kind="ExternalInput").ap()
    perm_ap = nc.dram_tensor("perm", [128, 128], f16, kind="ExternalInput").ap()
    mask_ap = nc.dram_tensor("mask4", [128, 4 * 512], f16,
                             kind="ExternalInput").ap()
    yT_ap = nc.dram_tensor("yT", [H, S], f16, kind="ExternalOutput").ap()

    with tile.TileContext(nc) as tc:
        from contextlib import ExitStack

        with ExitStack() as ctx:
            const = ctx.enter_context(tc.tile_pool(name="const", bufs=1))
            ident = const.tile([128, 128], f16, name="ident")
            make_identity(nc, ident[:])
            ones_col = const.tile([128, 1], f16, name="ones_col")
            nc.vector.memset(ones_col[:], 1.0)
            ones_sq = const.tile([128, 128], f16, name="ones_sq")
            nc.vector.memset(ones_sq[:], 1.0)
            nbias = const.tile([128, 1], f32, name="nbias")
            nc.vector.memset(nbias[:], -4.0)
            perm_sb = const.tile([128, 128], f16, name="perm_sb")
            cosI_sb = const.tile([128, S], f16, name="cosI_sb")
            sinI_sb = const.tile([128, S], f16, name="sinI_sb")
            mask_sb = const.tile([128, 4, 512], f16, name="mask_sb")

            # persistent activations
            res = ctx.enter_context(tc.tile_pool(name="res", bufs=1))
            qkT_sb = res.tile([128, 5, S], f16, name="qkT_sb")
            v_sb = res.tile([128, KT, 128], f16, name="v_sb")
            outT_sb = res.tile([128, NQ, S], f16, name="outT_sb")
            # head h -> (l-bank h//2, partition 64*(h%2)) rows
            rinvf = res.tile([128, 2, 512], f32, name="rinvf")
            rinv16 = res.tile([128, 2, 512], f16, name="rinv16")

            # released after the last qkv s-tile: w + streamed xT
            w_pool_cm = tc.tile_pool(name="w_pool", bufs=1, side="right")
            w_pool = w_pool_cm.__enter__()
            w_sb = w_pool.tile([128, KC, MQKV], f16, name="w_sb")
            xt_pool_cm = tc.tile_pool(name="xt_pool", bufs=2, side="right")
            xt_pool = xt_pool_cm.__enter__()

            # streaming SBUF pools
            stp = ctx.enter_context(tc.tile_pool(name="stp", bufs=3))
            ttp = ctx.enter_context(tc.tile_pool(name="ttp", bufs=4))
            ptp = ctx.enter_context(tc.tile_pool(name="ptp", bufs=4))
            yep = ctx.enter_context(tc.tile_pool(name="yep", bufs=6))

            # PSUM: 2 streaming banks + 6 held (4x O^T accum + 2 l banks)
            sp_ps = ctx.enter_context(
                tc.tile_pool(name="sp_ps", bufs=2, space="PSUM"))
            held_ps_cm = tc.tile_pool(name="held_ps", bufs=1, space="PSUM")
            held_ps = held_ps_cm.__enter__()

            # Input DMAs: SP and Activation each own a HWDGE queue, so
            # interleave w (SP) with xT (Act) in fine chunks; consts follow
            # the first-dependency chunks so the opening matmuls start early.
            w_src = w_ap.rearrange("(a p) m -> p a m", p=128)
            xT_src = xT_ap.rearrange("(a p) s -> p a s", p=128)

            def xT_load(g, eng):
                xt = xt_pool.tile([128, KC, 512], f16, tag="xt",
                                  name=f"xt{g}")
                for sub in range(4):
                    eng.dma_start(
                        out=xt[:, sub * 8:(sub + 1) * 8, :],
                        in_=xT_src[:, sub * 8:(sub + 1) * 8,
                                   g * 512:(g + 1) * 512])
                return xt

            xT_cur = xt_pool.tile([128, KC, 512], f16, tag="xt", name="xt0")
            for wc in range(8):
                nc.sync.dma_start(
                    out=w_sb[:, wc * 4:(wc + 1) * 4, :],
                    in_=w_src[:, wc * 4:(wc + 1) * 4, :])
                nc.scalar.dma_start(
                    out=xT_cur[:, wc * 4:(wc + 1) * 4, :],
                    in_=xT_src[:, wc * 4:(wc + 1) * 4, 0:512])
                if wc == 1:
                    nc.sync.dma_start(out=perm_sb[:], in_=perm_ap[:, :])
                    nc.scalar.dma_start(out=cosI_sb[:], in_=cosI_ap[:, :])
                    nc.sync.dma_start(out=sinI_sb[:], in_=sinI_ap[:, :])
                    nc.scalar.dma_start(out=mask_sb[:], in_=mask_ap[:, :])
            def normalize(g, lrow, Ops):
                # O^T /= l per head: 1/l broadcast along partitions via a
                # ones-column outer-product matmul
                for h in range(NQ):
                    p0 = 64 * (h % 2)
                    bk = h // 2
                    nc.vector.reciprocal(rinvf[p0:p0 + 1, bk, :], lrow[h])
                    nc.scalar.copy(out=rinv16[p0:p0 + 1, bk, :],
                                   in_=rinvf[p0:p0 + 1, bk, :])
                    bc = sp_ps.tile([128, 512], f32, tag="sp", name=f"bc{g}")
                    nc.tensor.matmul(
                        bc[:],
                        lhsT=ones_sq[p0:p0 + 1, :],
                        rhs=rinv16[p0:p0 + 1, bk, :],
                        start=True, stop=True)
                    bcs = ttp.tile([128, 512], f16, tag="t1", name=f"bcs{g}")
                    nc.scalar.copy(out=bcs[:], in_=bc[:])
                    nc.vector.tensor_mul(
                        outT_sb[:, h, g * 512:(g + 1) * 512],
                        Ops[h][:], bcs[:])

            pending = None
            for g in range(NG):
                if g + 1 < NG:
                    xT_next = xT_load(g + 1,
                                      nc.scalar if g % 2 else nc.sync)

                # ---- qkv^T projection for s-tile g (6 chunks of 128) ----
                for n in range(6):
                    ps = sp_ps.tile([128, 512], f32, tag="sp")
                    for kc in range(KC):
                        nc.tensor.matmul(
                            ps[:],
                            lhsT=w_sb[:, kc, n * 128:(n + 1) * 128],
                            rhs=xT_cur[:, kc, :],
                            start=(kc == 0), stop=(kc == KC - 1))
                    stage = stp.tile([128, 512], f16, tag="stage")
                    nc.scalar.copy(out=stage[:], in_=ps[:])
                    if n < 5:
                        # rope: pair-swap via PE perm matmul + 3 DVE ops
                        sw = sp_ps.tile([128, 512], f32, tag="sp")
                        nc.tensor.matmul(sw[:], lhsT=perm_sb[:],
                                         rhs=stage[:], start=True, stop=True)
                        t1 = ttp.tile([128, 512], f16, tag="t1")
                        nc.vector.tensor_mul(
                            t1[:], stage[:], cosI_sb[:, g * 512:(g + 1) * 512])
                        t2 = ttp.tile([128, 512], f16, tag="t2")
                        nc.vector.tensor_mul(
                            t2[:], sw[:], sinI_sb[:, g * 512:(g + 1) * 512])
                        nc.vector.tensor_add(
                            qkT_sb[:, n, g * 512:(g + 1) * 512], t1[:], t2[:])
                    else:
                        # v: transpose chunks into natural [s, d] layout
                        for c4 in range(4):
                            vps = sp_ps.tile([128, 128], f16, tag="sp")
                            nc.tensor.transpose(
                                vps[:], stage[:, c4 * 128:(c4 + 1) * 128],
                                ident[:])
                            nc.scalar.copy(out=v_sb[:, g * 4 + c4, :],
                                           in_=vps[:])

                if g == NG - 1:
                    # xT/w dead after the last projection: swap in wo so its
                    # DMA overlaps the last attention group
                    xt_pool_cm.__exit__(None, None, None)
                    w_pool_cm.__exit__(None, None, None)
                    wo_pool = ctx.enter_context(
                        tc.tile_pool(name="wo_pool", bufs=1, side="right"))
                    wo_sb = wo_pool.tile([128, NQ, H], f16, name="wo_sb")
                    wo_src = wo_ap.rearrange("(a p) m -> p a m", p=128)
                    for hc in range(4):
                        nc.scalar.dma_start(
                            out=wo_sb[:, :, hc * 1024:(hc + 1) * 1024],
                            in_=wo_src[:, :, hc * 1024:(hc + 1) * 1024])

                # normalize of the previous group overlaps this projection
                if pending is not None:
                    normalize(*pending)

                # ---- attention for q-group g ----
                nkc = 4 * g + 4   # causal k-chunks for this group
                lA = held_ps.tile([128, 512], f32, tag="lA")
                lB = held_ps.tile([128, 512], f32, tag="lB")
                lrow = [lA[0:1, :], lA[64:65, :], lB[0:1, :], lB[64:65, :]]
                Ops = [held_ps.tile([128, 512], f32, tag=f"O{h}",
                                    name=f"O{g}_{h}")
                       for h in range(NQ)]
                for jj in range(nkc):
                    PT = ptp.tile([128, NQ, 512], f16, tag="PT")
                    for h in range(NQ):
                        sps = sp_ps.tile([128, 512], f32, tag="sp")
                        nc.tensor.matmul(
                            sps[:],
                            lhsT=qkT_sb[:, 4, jj * 128:(jj + 1) * 128],
                            rhs=qkT_sb[:, h, g * 512:(g + 1) * 512],
                            start=True, stop=True)
                        nc.scalar.activation(PT[:, h, :], sps[:], Exp,
                                             scale=SCALE, bias=nbias[:])
                        if jj >= 4 * g:
                            nc.vector.tensor_mul(
                                PT[:, h, :], PT[:, h, :],
                                mask_sb[:, jj - 4 * g, :])
                        nc.tensor.matmul(
                            lrow[h],
                            lhsT=ones_col[:],
                            rhs=PT[:, h, :],
                            start=(jj == 0), stop=(jj == nkc - 1))
                        nc.tensor.matmul(
                            Ops[h][:],
                            lhsT=v_sb[:, jj, :],
                            rhs=PT[:, h, :],
                            start=(jj == 0), stop=(jj == nkc - 1))
                pending = (g, lrow, Ops)
                xT_cur = xT_next if g + 1 < NG else None

            normalize(*pending)

            # ---- release the held PSUM banks for the y-proj rotation ----
            held_ps_cm.__exit__(None, None, None)
            y_ps = ctx.enter_context(
                tc.tile_pool(name="y_ps", bufs=4, space="PSUM"))

            # ---- output projection y^T = wo^T @ O^T ----
            for g in range(NG):
                for ym in range(H // 128):
                    yps = y_ps.tile([128, 512], f32, tag="y")
                    for kc in range(NQ):
                        nc.tensor.matmul(
                            yps[:],
                            lhsT=wo_sb[:, kc, ym * 128:(ym + 1) * 128],
                            rhs=outT_sb[:, kc, g * 512:(g + 1) * 512],
                            start=(kc == 0), stop=(kc == NQ - 1))
                    ye = yep.tile([128, 512], f16, tag="ye")
                    if ym % 2:
                        nc.scalar.copy(out=ye[:], in_=yps[:])
                    else:
                        nc.vector.tensor_copy(out=ye[:], in_=yps[:])
                    (nc.sync if ym % 2 else nc.scalar).dma_start(
                        out=yT_ap[ym * 128:(ym + 1) * 128,
                                  g * 512:(g + 1) * 512],
                        in_=ye[:])

    nc.compile()
    return nc


def _get_nc():
    if "nc" not in _CACHE:
        _CACHE["nc"] = _build()
    return _CACHE["nc"]


def _host_consts(rope_cache):
    rc = np.asarray(rope_cache, np.float32)          # [S, 64, 2]
    cos = rc[:, :, 0].T                              # [64, S]
    sin = rc[:, :, 1].T
    cosI = np.repeat(cos, 2, axis=0).astype(np.float16)   # [128, S]
    sinI = np.repeat(sin, 2, axis=0)
    sinI[0::2] *= -1.0                               # even rows: -sin
    sinI = sinI.astype(np.float16)

    idx = np.arange(128)
    perm = np.zeros((128, 128), np.float16)
    perm[idx ^ 1, idx] = 1.0                         # out[p] = in[p^1]

    kp = np.arange(128)[:, None]
    qf = np.arange(128)[None, :]
    tri = (qf >= kp)                                 # [128, 128] visible
    mask4 = np.zeros((128, 4, 4, 128), np.float16)   # [kp, r, c, qf]
    for r in range(4):
        for c in range(4):
            if c > r:
                mask4[:, r, c, :] = 1.0
            elif c == r:
                mask4[:, r, c, :] = tri
    mask4 = mask4.reshape(128, 4 * 512)
    return cosI, sinI, perm, mask4


def kernel(x, last_pos, mask, rope_cache, wqkv, wo):
    global LAST_RESULTS
    from concourse.bass_utils import run_bass_kernel_spmd

    nc = _get_nc()

    x2 = np.asarray(x, np.float32).reshape(S, H)
    xT16 = np.ascontiguousarray(x2.T.astype(np.float16))
    wq = np.asarray(wqkv, np.float32)
    wo_f = np.asarray(wo, np.float32)
    cosI, sinI, perm, mask4 = _host_consts(rope_cache)

    in_maps = []
    for c in range(N_CORES):
        wcat = np.concatenate(
            [wq[:, c * 512:(c + 1) * 512],
             wq[:, H + c * 128:H + (c + 1) * 128],
             wq[:, H + 1024 + c * 128:H + 1024 + (c + 1) * 128]],
            axis=1).astype(np.float16)
        in_maps.append({
            "xT": xT16,
            "w": np.ascontiguousarray(wcat),
            "wo": np.ascontiguousarray(
                wo_f[c * 512:(c + 1) * 512, :].astype(np.float16)),
            "cosI": cosI,
            "sinI": sinI,
            "perm": perm,
            "mask4": mask4,
        })

    res = run_bass_kernel_spmd(nc, in_maps, list(range(N_CORES)))
    LAST_RESULTS = res
    if res.exec_time_ns is not None:
        print(f"HW exec time: {res.exec_time_ns} ns")
    yT = res.results[0]["yT"].astype(np.float32)
    for c in range(1, N_CORES):
        yT = yT + res.results[c]["yT"].astype(np.float32)
    return np.ascontiguousarray(yT.T).reshape(1, S, H).astype(np.float32)


# revision 27
# speedup vs baseline: 1.2375x; 1.0312x over previous
"""Llama3 attention prefill kernel for 8 Trainium2 NeuronCores — v2.

Sharding: tensor-parallel over heads. Core c owns Q heads 4c..4c+3 and KV
head c (GQA group), plus the matching wqkv columns / wo rows. Each core
computes a partial output y_c = attn_c @ wo_c; the host sums the partials.

v2 design (driven by the TimelineSim cost model):
  * PE sequencer cost (~167ns/matmul) dominated v1 (3376 PE instrs), so v2
    issues ~1800 larger matmuls instead.
  * Host supplies x^T in f16, so the QKV projection runs in transposed
    layout (out = w_chunk^T @ x^T = qkv^T) with zero x-transposes and
    produces Q^T/K^T/V^T directly.
  * RoPE runs on the transposed q/k chunks: the pair-swap is one PE
    permutation matmul per chunk (host-provided swap matrix), then 3 DVE
    elementwise ops with host-precomputed interleaved cos / +-sin rows.
  * Attention computes scores TRANSPOSED (S^T[k,q] = K Q^T) per q-group of
    512 so exp (Act) writes P^T straight to SBUF — no P transposes, no
    PSUM->SBUF P evictions. Softmax denominators come from ones-vector
    matmuls accumulated in PSUM; normalization happens after PV on the
    [128,512] O^T tile via a broadcast outer-product matmul.
  * All weights/activations f16 on the wire (host pre-casts), f32 PSUM
    accumulation everywhere.
"""

import os
import sys

for _p in ("/opt/trn_rl_repo", "/root/.axon_site/_ro/trn_rl_repo"):
    if os.path.isdir(_p) and _p not in sys.path:
        sys.path.insert(0, _p)

import numpy as np

S = 2048
H = 4096
HD = 128
NQ = 4            # q heads per core
MQKV = 768        # per-core qkv columns: 512 q + 128 k + 128 v
N_CORES = 8
SCALE = 1.0 / float(np.sqrt(HD))
KC = H // 128     # 32 contraction chunks for qkv
NG = 4            # q-groups of 512
KT = S // 128     # 16 k-tiles

_CACHE = {}
LAST_RESULTS = None


def _build():
    import concourse.tile as tile
    from concourse import bacc, mybir
    from concourse.masks import make_identity

    f32 = mybir.dt.float32
    f16 = mybir.dt.float16
    Exp = mybir.ActivationFunctionType.Exp

    nc = bacc.Bacc("TRN2", target_bir_lowering=False, debug=False)

    xT_ap = nc.dram_tensor("xT", [H, S], f16, kind="ExternalInput").ap()
    w_ap = nc.dram_tensor("w", [H, MQKV], f16, kind="ExternalInput").ap()
    wo_ap = nc.dram_tensor("wo", [NQ * HD, H], f16, kind="ExternalInput").ap()
    cosI_ap = nc.dram_tensor("cosI", [128, S], f16, kind="ExternalInput").ap()
    sinI_ap = nc.dram_tensor("sinI", [128, S], f16, kind="ExternalInput").ap()
    perm_ap = nc.dram_tensor("perm", [128, 128], f16, kind="ExternalInput").ap()
    mask_ap = nc.dram_tensor("mask4", [128, 4 * 512], f16,
                             kind="ExternalInput").ap()
    yT_ap = nc.dram_tensor("yT", [H, S], f16, kind="ExternalOutput").ap()

    with tile.TileContext(nc) as tc:
        from contextlib import ExitStack

        with ExitStack() as ctx:
            const = ctx.enter_context(tc.tile_pool(name="const", bufs=1))
            ident = const.tile([128, 128], f16, name="ident")
            make_identity(nc, ident[:])
            ones_col = const.tile([128, 1], f16, name="ones_col")
            nc.vector.memset(ones_col[:], 1.0)
            ones_sq = const.tile([128, 128], f16, name="ones_sq")
            nc.vector.memset(ones_sq[:], 1.0)
            nbias = const.tile([128, 1], f32, name="nbias")
            nc.vector.memset(nbias[:], -4.0)
            perm_sb = const.tile([128, 128], f16, name="perm_sb")
            cosI_sb = const.tile([128, S], f16, name="cosI_sb")
            sinI_sb = const.tile([128, S], f16, name="sinI_sb")
            mask_sb = const.tile([128, 4, 512], f16, name="mask_sb")

            # persistent activations
            res = ctx.enter_context(tc.tile_pool(name="res", bufs=1))
            qkT_sb = res.tile([128, 5, S], f16, name="qkT_sb")
            v_sb = res.tile([128, KT, 128], f16, name="v_sb")
            outT_sb = res.tile([128, NQ, S], f16, name="outT_sb")
            # head h -> (l-bank h//2, partition 64*(h%2)) rows
            rinvf = res.tile([128, 2, 512], f32, name="rinvf")
            rinv16 = res.tile([128, 2, 512], f16, name="rinv16")

            # released after the last qkv s-tile: w + streamed xT
            w_pool_cm = tc.tile_pool(name="w_pool", bufs=1, side="right")
            w_pool = w_pool_cm.__enter__()
            w_sb = w_pool.tile([128, KC, MQKV], f16, name="w_sb")
            xt_pool_cm = tc.tile_pool(name="xt_pool", bufs=2, side="right")
            xt_pool = xt_pool_cm.__enter__()

            # streaming SBUF pools
            stp = ctx.enter_context(tc.tile_pool(name="stp", bufs=3))
            ttp = ctx.enter_context(tc.tile_pool(name="ttp", bufs=4))
            ptp = ctx.enter_context(tc.tile_pool(name="ptp", bufs=4))
            yep = ctx.enter_context(tc.tile_pool(name="yep", bufs=6))

            # PSUM: 2 streaming banks + 6 held (4x O^T accum + 2 l banks)
            sp_ps = ctx.enter_context(
                tc.tile_pool(name="sp_ps", bufs=2, space="PSUM"))
            held_ps_cm = tc.tile_pool(name="held_ps", bufs=1, space="PSUM")
            held_ps = held_ps_cm.__enter__()

            # Input DMAs: SP and Activation each own a HWDGE queue, so
            # interleave w (SP) with xT (Act) in fine chunks; consts follow
            # the first-dependency chunks so the opening matmuls start early.
            w_src = w_ap.rearrange("(a p) m -> p a m", p=128)
            xT_src = xT_ap.rearrange("(a p) s -> p a s", p=128)

            def xT_load(g, eng):
                xt = xt_pool.tile([128, KC, 512], f16, tag="xt",
                                  name=f"xt{g}")
                for sub in range(4):
                    eng.dma_start(
                        out=xt[:, sub * 8:(sub + 1) * 8, :],
                        in_=xT_src[:, sub * 8:(sub + 1) * 8,
                                   g * 512:(g + 1) * 512])
                return xt

            xT_cur = xt_pool.tile([128, KC, 512], f16, tag="xt", name="xt0")
            for wc in range(8):
                nc.sync.dma_start(
                    out=w_sb[:, wc * 4:(wc + 1) * 4, :],
                    in_=w_src[:, wc * 4:(wc + 1) * 4, :])
                nc.scalar.dma_start(
                    out=xT_cur[:, wc * 4:(wc + 1) * 4, :],
                    in_=xT_src[:, wc * 4:(wc + 1) * 4, 0:512])
                if wc == 1:
                    nc.sync.dma_start(out=perm_sb[:], in_=perm_ap[:, :])
                    nc.scalar.dma_start(out=cosI_sb[:], in_=cosI_ap[:, :])
                    nc.sync.dma_start(out=sinI_sb[:], in_=sinI_ap[:, :])
                    nc.scalar.dma_start(out=mask_sb[:], in_=mask_ap[:, :])
            def normalize(g, lrow, Ops):
                # O^T /= l per head: 1/l broadcast along partitions via a
                # ones-column outer-product matmul
                for h in range(NQ):
                    p0 = 64 * (h % 2)
                    bk = h // 2
                    nc.vector.reciprocal(rinvf[p0:p0 + 1, bk, :], lrow[h])
                    nc.scalar.copy(out=rinv16[p0:p0 + 1, bk, :],
                                   in_=rinvf[p0:p0 + 1, bk, :])
                    bc = sp_ps.tile([128, 512], f32, tag="sp", name=f"bc{g}")
                    nc.tensor.matmul(
                        bc[:],
                        lhsT=ones_sq[p0:p0 + 1, :],
                        rhs=rinv16[p0:p0 + 1, bk, :],
                        start=True, stop=True)
                    bcs = ttp.tile([128, 512], f16, tag="t1", name=f"bcs{g}")
                    nc.scalar.copy(out=bcs[:], in_=bc[:])
                    nc.vector.tensor_mul(
                        outT_sb[:, h, g * 512:(g + 1) * 512],
                        Ops[h][:], bcs[:])

            def finish_chunk(g, n, ps):
                stage = stp.tile([128, 512], f16, tag="stage")
                nc.scalar.copy(out=stage[:], in_=ps[:])
                if n < 5:
                    # rope: pair-swap via PE perm matmul + 3 DVE ops
                    sw = sp_ps.tile([128, 512], f32, tag="sp", name="sw")
                    nc.tensor.matmul(sw[:], lhsT=perm_sb[:],
                                     rhs=stage[:], start=True, stop=True)
                    t1 = ttp.tile([128, 512], f16, tag="t1")
                    nc.vector.tensor_mul(
                        t1[:], stage[:], cosI_sb[:, g * 512:(g + 1) * 512])
                    t2 = ttp.tile([128, 512], f16, tag="t2")
                    nc.vector.tensor_mul(
                        t2[:], sw[:], sinI_sb[:, g * 512:(g + 1) * 512])
                    nc.vector.tensor_add(
                        qkT_sb[:, n, g * 512:(g + 1) * 512], t1[:], t2[:])
                else:
                    # v: transpose chunks into natural [s, d] layout
                    for c4 in range(4):
                        vps = sp_ps.tile([128, 128], f16, tag="sp",
                                         name="vps")
                        nc.tensor.transpose(
                            vps[:], stage[:, c4 * 128:(c4 + 1) * 128],
                            ident[:])
                        nc.scalar.copy(out=v_sb[:, g * 4 + c4, :],
                                       in_=vps[:])

            pending = None
            for g in range(NG):
                if g + 1 < NG:
                    xT_next = xT_load(g + 1,
                                      nc.scalar if g % 2 else nc.sync)

                # ---- qkv^T projection for s-tile g (6 chunks of 128),
                # paired so two accumulations interleave per kc (keeps PE
                # fed while the startup DMAs stream in) ----
                def project_pair(na, nb):
                    psA = sp_ps.tile([128, 512], f32, tag="sp", name="psA")
                    psB = sp_ps.tile([128, 512], f32, tag="sp", name="psB")
                    for kc in range(KC):
                        for n, ps in ((na, psA), (nb, psB)):
                            nc.tensor.matmul(
                                ps[:],
                                lhsT=w_sb[:, kc, n * 128:(n + 1) * 128],
                                rhs=xT_cur[:, kc, :],
                                start=(kc == 0), stop=(kc == KC - 1))
                    return psA, psB

                for na in range(0, 6, 2):
                    pair = project_pair(na, na + 1)
                    for n, ps in zip((na, na + 1), pair):
                        finish_chunk(g, n, ps)

                if g == NG - 1:
                    # xT/w dead after the last projection: swap in wo so its
                    # DMA overlaps the last attention group
                    xt_pool_cm.__exit__(None, None, None)
                    w_pool_cm.__exit__(None, None, None)
                    wo_pool = ctx.enter_context(
                        tc.tile_pool(name="wo_pool", bufs=1, side="right"))
                    wo_sb = wo_pool.tile([128, NQ, H], f16, name="wo_sb")
                    wo_src = wo_ap.rearrange("(a p) m -> p a m", p=128)
                    for hc in range(4):
                        nc.scalar.dma_start(
                            out=wo_sb[:, :, hc * 1024:(hc + 1) * 1024],
                            in_=wo_src[:, :, hc * 1024:(hc + 1) * 1024])

                # normalize of the previous group overlaps this projection
                if pending is not None:
                    normalize(*pending)

                # ---- attention for q-group g ----
                nkc = 4 * g + 4   # causal k-chunks for this group
                lA = held_ps.tile([128, 512], f32, tag="lA")
                lB = held_ps.tile([128, 512], f32, tag="lB")
                lrow = [lA[0:1, :], lA[64:65, :], lB[0:1, :], lB[64:65, :]]
                Ops = [held_ps.tile([128, 512], f32, tag=f"O{h}",
                                    name=f"O{g}_{h}")
                       for h in range(NQ)]
                for jj in range(nkc):
                    PT = ptp.tile([128, NQ, 512], f16, tag="PT")
                    # wave 1: all scores + exp first, so the l/PV matmuls
                    # (which wait on exp) never block the in-order PE queue
                    for h in range(NQ):
                        sps = sp_ps.tile([128, 512], f32, tag="sp")
                        nc.tensor.matmul(
                            sps[:],
                            lhsT=qkT_sb[:, 4, jj * 128:(jj + 1) * 128],
                            rhs=qkT_sb[:, h, g * 512:(g + 1) * 512],
                            start=True, stop=True)
                        nc.scalar.activation(PT[:, h, :], sps[:], Exp,
                                             scale=SCALE, bias=nbias[:])
                        if jj >= 4 * g:
                            nc.vector.tensor_mul(
                                PT[:, h, :], PT[:, h, :],
                                mask_sb[:, jj - 4 * g, :])
                    # wave 2: accumulate denominators and PV
                    for h in range(NQ):
                        nc.tensor.matmul(
                            lrow[h],
                            lhsT=ones_col[:],
                            rhs=PT[:, h, :],
                            start=(jj == 0), stop=(jj == nkc - 1))
                        nc.tensor.matmul(
                            Ops[h][:],
                            lhsT=v_sb[:, jj, :],
                            rhs=PT[:, h, :],
                            start=(jj == 0), stop=(jj == nkc - 1))
                pending = (g, lrow, Ops)
                xT_cur = xT_next if g + 1 < NG else None

            normalize(*pending)

            # ---- release the held PSUM banks for the y-proj rotation ----
            held_ps_cm.__exit__(None, None, None)
            y_ps = ctx.enter_context(
                tc.tile_pool(name="y_ps", bufs=4, space="PSUM"))

            # ---- output projection y^T = wo^T @ O^T ----
            for g in range(NG):
                for ym in range(H // 128):
                    yps = y_ps.tile([128, 512], f32, tag="y")
                    for kc in range(NQ):
                        nc.tensor.matmul(
                            yps[:],
                            lhsT=wo_sb[:, kc, ym * 128:(ym + 1) * 128],
                            rhs=outT_sb[:, kc, g * 512:(g + 1) * 512],
                            start=(kc == 0), stop=(kc == NQ - 1))
                    ye = yep.tile([128, 512], f16, tag="ye")
                    if ym % 2:
                        nc.scalar.copy(out=ye[:], in_=yps[:])
                    else:
                        nc.vector.tensor_copy(out=ye[:], in_=yps[:])
                    (nc.sync if ym % 2 else nc.scalar).dma_start(
                        out=yT_ap[ym * 128:(ym + 1) * 128,
                                  g * 512:(g + 1) * 512],
                        in_=ye[:])

    nc.compile()
    return nc


def _get_nc():
    if "nc" not in _CACHE:
        _CACHE["nc"] = _build()
    return _CACHE["nc"]


def _host_consts(rope_cache):
    rc = np.asarray(rope_cache, np.float32)          # [S, 64, 2]
    cos = rc[:, :, 0].T                              # [64, S]
    sin = rc[:, :, 1].T
    cosI = np.repeat(cos, 2, axis=0).astype(np.float16)   # [128, S]
    sinI = np.repeat(sin, 2, axis=0)
    sinI[0::2] *= -1.0                               # even rows: -sin
    sinI = sinI.astype(np.float16)

    idx = np.arange(128)
    perm = np.zeros((128, 128), np.float16)
    perm[idx ^ 1, idx] = 1.0                         # out[p] = in[p^1]

    kp = np.arange(128)[:, None]
    qf = np.arange(128)[None, :]
    tri = (qf >= kp)                                 # [128, 128] visible
    mask4 = np.zeros((128, 4, 4, 128), np.float16)   # [kp, r, c, qf]
    for r in range(4):
        for c in range(4):
            if c > r:
                mask4[:, r, c, :] = 1.0
            elif c == r:
                mask4[:, r, c, :] = tri
    mask4 = mask4.reshape(128, 4 * 512)
    return cosI, sinI, perm, mask4


def kernel(x, last_pos, mask, rope_cache, wqkv, wo):
    global LAST_RESULTS
    from concourse.bass_utils import run_bass_kernel_spmd

    nc = _get_nc()

    x2 = np.asarray(x, np.float32).reshape(S, H)
    xT16 = np.ascontiguousarray(x2.T.astype(np.float16))
    wq = np.asarray(wqkv, np.float32)
    wo_f = np.asarray(wo, np.float32)
    cosI, sinI, perm, mask4 = _host_consts(rope_cache)

    in_maps = []
    for c in range(N_CORES):
        wcat = np.concatenate(
            [wq[:, c * 512:(c + 1) * 512],
             wq[:, H + c * 128:H + (c + 1) * 128],
             wq[:, H + 1024 + c * 128:H + 1024 + (c + 1) * 128]],
            axis=1).astype(np.float16)
        in_maps.append({
            "xT": xT16,
            "w": np.ascontiguousarray(wcat),
            "wo": np.ascontiguousarray(
                wo_f[c * 512:(c + 1) * 512, :].astype(np.float16)),
            "cosI": cosI,
            "sinI": sinI,
            "perm": perm,
            "mask4": mask4,
        })

    res = run_bass_kernel_spmd(nc, in_maps, list(range(N_CORES)))
    LAST_RESULTS = res
    if res.exec_time_ns is not None:
        print(f"HW exec time: {res.exec_time_ns} ns")
    yT = res.results[0]["yT"].astype(np.float32)
    for c in range(1, N_CORES):
        yT = yT + res.results[c]["yT"].astype(np.float32)
    return np.ascontiguousarray(yT.T).reshape(1, S, H).astype(np.float32)


# revision 29
# speedup vs baseline: 1.2608x; 1.0189x over previous
"""Llama3 attention prefill kernel for 8 Trainium2 NeuronCores — v2.

Sharding: tensor-parallel over heads. Core c owns Q heads 4c..4c+3 and KV
head c (GQA group), plus the matching wqkv columns / wo rows. Each core
computes a partial output y_c = attn_c @ wo_c; the host sums the partials.

v2 design (driven by the TimelineSim cost model):
  * PE sequencer cost (~167ns/matmul) dominated v1 (3376 PE instrs), so v2
    issues ~1800 larger matmuls instead.
  * Host supplies x^T in f16, so the QKV projection runs in transposed
    layout (out = w_chunk^T @ x^T = qkv^T) with zero x-transposes and
    produces Q^T/K^T/V^T directly.
  * RoPE runs on the transposed q/k chunks: the pair-swap is one PE
    permutation matmul per chunk (host-provided swap matrix), then 3 DVE
    elementwise ops with host-precomputed interleaved cos / +-sin rows.
  * Attention computes scores TRANSPOSED (S^T[k,q] = K Q^T) per q-group of
    512 so exp (Act) writes P^T straight to SBUF — no P transposes, no
    PSUM->SBUF P evictions. Softmax denominators come from ones-vector
    matmuls accumulated in PSUM; normalization happens after PV on the
    [128,512] O^T tile via a broadcast outer-product matmul.
  * All weights/activations f16 on the wire (host pre-casts), f32 PSUM
    accumulation everywhere.
"""

import os
import sys

for _p in ("/opt/trn_rl_repo", "/root/.axon_site/_ro/trn_rl_repo"):
    if os.path.isdir(_p) and _p not in sys.path:
        sys.path.insert(0, _p)

import numpy as np

S = 2048
H = 4096
HD = 128
NQ = 4            # q heads per core
MQKV = 768        # per-core qkv columns: 512 q + 128 k + 128 v
N_CORES = 8
SCALE = 1.0 / float(np.sqrt(HD))
KC = H // 128     # 32 contraction chunks for qkv
NG = 4            # q-groups of 512
KT = S // 128     # 16 k-tiles

_CACHE = {}
LAST_RESULTS = None


def _build():
    import concourse.tile as tile
    from concourse import bacc, mybir
    from concourse.masks import make_identity

    f32 = mybir.dt.float32
    f16 = mybir.dt.float16
    Exp = mybir.ActivationFunctionType.Exp

    nc = bacc.Bacc("TRN2", target_bir_lowering=False, debug=False)

    xT_ap = nc.dram_tensor("xT", [H, S], f16, kind="ExternalInput").ap()
    w_ap = nc.dram_tensor("w", [H, MQKV], f16, kind="ExternalInput").ap()
    wo_ap = nc.dram_tensor("wo", [NQ * HD, H], f16, kind="ExternalInput").ap()
    cosI_ap = nc.dram_tensor("cosI", [128, S], f16, kind="ExternalInput").ap()
    sinI_ap = nc.dram_tensor("sinI", [128, S], f16, kind="ExternalInput").ap()
    perm_ap = nc.dram_tensor("perm", [128, 128], f16, kind="ExternalInput").ap()
    mask_ap = nc.dram_tensor("mask4", [128, 4 * 512], f16,
                             kind="ExternalInput").ap()
    yT_ap = nc.dram_tensor("yT", [H, S], f16, kind="ExternalOutput").ap()

    with tile.TileContext(nc) as tc:
        from contextlib import ExitStack

        with ExitStack() as ctx:
            const = ctx.enter_context(tc.tile_pool(name="const", bufs=1))
            ident = const.tile([128, 128], f16, name="ident")
            make_identity(nc, ident[:])
            ones_col = const.tile([128, 1], f16, name="ones_col")
            nc.vector.memset(ones_col[:], 1.0)
            ones_sq = const.tile([128, 128], f16, name="ones_sq")
            nc.vector.memset(ones_sq[:], 1.0)
            nbias = const.tile([128, 1], f32, name="nbias")
            nc.vector.memset(nbias[:], -4.0)
            perm_sb = const.tile([128, 128], f16, name="perm_sb")
            cosI_sb = const.tile([128, S], f16, name="cosI_sb")
            sinI_sb = const.tile([128, S], f16, name="sinI_sb")
            mask_sb = const.tile([128, 4, 512], f16, name="mask_sb")

            # persistent activations
            res = ctx.enter_context(tc.tile_pool(name="res", bufs=1))
            qkT_sb = res.tile([128, 5, S], f16, name="qkT_sb")
            v_sb = res.tile([128, KT, 128], f16, name="v_sb")
            outT_sb = res.tile([128, NQ, S], f16, name="outT_sb")
            # head h -> (l-bank h//2, partition 64*(h%2)) rows
            rinvf = res.tile([128, 2, 512], f32, name="rinvf")
            rinv16 = res.tile([128, 2, 512], f16, name="rinv16")

            # released after the last qkv s-tile: w + streamed xT
            w_pool_cm = tc.tile_pool(name="w_pool", bufs=1, side="right")
            w_pool = w_pool_cm.__enter__()
            w_sb = w_pool.tile([128, KC, MQKV], f16, name="w_sb")
            xt_pool_cm = tc.tile_pool(name="xt_pool", bufs=2, side="right")
            xt_pool = xt_pool_cm.__enter__()

            # streaming SBUF pools
            stp = ctx.enter_context(tc.tile_pool(name="stp", bufs=3))
            ttp = ctx.enter_context(tc.tile_pool(name="ttp", bufs=4))
            ptp = ctx.enter_context(tc.tile_pool(name="ptp", bufs=4))
            yep = ctx.enter_context(tc.tile_pool(name="yep", bufs=6))

            # PSUM: 2 streaming banks + 6 held (4x O^T accum + 2 l banks)
            sp_ps = ctx.enter_context(
                tc.tile_pool(name="sp_ps", bufs=2, space="PSUM"))
            held_ps_cm = tc.tile_pool(name="held_ps", bufs=1, space="PSUM")
            held_ps = held_ps_cm.__enter__()

            # Input DMAs: SP and Activation each own a HWDGE queue, so
            # interleave w (SP) with xT (Act) in fine chunks; consts follow
            # the first-dependency chunks so the opening matmuls start early.
            w_src = w_ap.rearrange("(a p) m -> p a m", p=128)
            xT_src = xT_ap.rearrange("(a p) s -> p a s", p=128)

            def xT_load(g, eng):
                xt = xt_pool.tile([128, KC, 512], f16, tag="xt",
                                  name=f"xt{g}")
                for sub in range(4):
                    eng.dma_start(
                        out=xt[:, sub * 8:(sub + 1) * 8, :],
                        in_=xT_src[:, sub * 8:(sub + 1) * 8,
                                   g * 512:(g + 1) * 512])
                return xt

            # startup: balance w + xT(0) across both HWDGE queues in
            # kc-need order; consts follow (their consumers are off the PE
            # critical path)
            xT_cur = xt_pool.tile([128, KC, 512], f16, tag="xt", name="xt0")
            for wc in range(8):
                ew = nc.sync if wc % 2 == 0 else nc.scalar
                ex = nc.scalar if wc % 2 == 0 else nc.sync
                ew.dma_start(
                    out=w_sb[:, wc * 4:(wc + 1) * 4, :],
                    in_=w_src[:, wc * 4:(wc + 1) * 4, :])
                ex.dma_start(
                    out=xT_cur[:, wc * 4:(wc + 1) * 4, :],
                    in_=xT_src[:, wc * 4:(wc + 1) * 4, 0:512])
            nc.sync.dma_start(out=perm_sb[:], in_=perm_ap[:, :])
            nc.scalar.dma_start(out=cosI_sb[:], in_=cosI_ap[:, :])
            nc.sync.dma_start(out=sinI_sb[:], in_=sinI_ap[:, :])
            nc.scalar.dma_start(out=mask_sb[:], in_=mask_ap[:, :])
            def normalize(g, lrow, Ops):
                # O^T /= l per head: 1/l broadcast along partitions via a
                # ones-column outer-product matmul
                for h in range(NQ):
                    p0 = 64 * (h % 2)
                    bk = h // 2
                    nc.vector.reciprocal(rinvf[p0:p0 + 1, bk, :], lrow[h])
                    nc.scalar.copy(out=rinv16[p0:p0 + 1, bk, :],
                                   in_=rinvf[p0:p0 + 1, bk, :])
                    bc = sp_ps.tile([128, 512], f32, tag="sp", name=f"bc{g}")
                    nc.tensor.matmul(
                        bc[:],
                        lhsT=ones_sq[p0:p0 + 1, :],
                        rhs=rinv16[p0:p0 + 1, bk, :],
                        start=True, stop=True)
                    bcs = ttp.tile([128, 512], f16, tag="t1", name=f"bcs{g}")
                    nc.scalar.copy(out=bcs[:], in_=bc[:])
                    nc.vector.tensor_mul(
                        outT_sb[:, h, g * 512:(g + 1) * 512],
                        Ops[h][:], bcs[:])

            def finish_chunk(g, n, ps):
                stage = stp.tile([128, 512], f16, tag="stage")
                nc.scalar.copy(out=stage[:], in_=ps[:])
                if n < 5:
                    # rope: pair-swap via PE perm matmul + 3 DVE ops
                    sw = sp_ps.tile([128, 512], f32, tag="sp", name="sw")
                    nc.tensor.matmul(sw[:], lhsT=perm_sb[:],
                                     rhs=stage[:], start=True, stop=True)
                    t1 = ttp.tile([128, 512], f16, tag="t1")
                    nc.vector.tensor_mul(
                        t1[:], stage[:], cosI_sb[:, g * 512:(g + 1) * 512])
                    t2 = ttp.tile([128, 512], f16, tag="t2")
                    nc.vector.tensor_mul(
                        t2[:], sw[:], sinI_sb[:, g * 512:(g + 1) * 512])
                    nc.vector.tensor_add(
                        qkT_sb[:, n, g * 512:(g + 1) * 512], t1[:], t2[:])
                else:
                    # v: transpose chunks into natural [s, d] layout
                    for c4 in range(4):
                        vps = sp_ps.tile([128, 128], f16, tag="sp",
                                         name="vps")
                        nc.tensor.transpose(
                            vps[:], stage[:, c4 * 128:(c4 + 1) * 128],
                            ident[:])
                        nc.scalar.copy(out=v_sb[:, g * 4 + c4, :],
                                       in_=vps[:])

            pending = None
            for g in range(NG):
                if g + 1 < NG:
                    xT_next = xT_load(g + 1,
                                      nc.scalar if g % 2 else nc.sync)

                # ---- qkv^T projection for s-tile g (6 chunks of 128),
                # paired so two accumulations interleave per kc (keeps PE
                # fed while the startup DMAs stream in) ----
                def project_pair(na, nb):
                    psA = sp_ps.tile([128, 512], f32, tag="sp", name="psA")
                    psB = sp_ps.tile([128, 512], f32, tag="sp", name="psB")
                    for kc in range(KC):
                        for n, ps in ((na, psA), (nb, psB)):
                            nc.tensor.matmul(
                                ps[:],
                                lhsT=w_sb[:, kc, n * 128:(n + 1) * 128],
                                rhs=xT_cur[:, kc, :],
                                start=(kc == 0), stop=(kc == KC - 1))
                    return psA, psB

                for na in range(0, 6, 2):
                    pair = project_pair(na, na + 1)
                    for n, ps in zip((na, na + 1), pair):
                        finish_chunk(g, n, ps)

                if g == NG - 1:
                    # xT/w dead after the last projection: swap in wo so its
                    # DMA overlaps the last attention group
                    xt_pool_cm.__exit__(None, None, None)
                    w_pool_cm.__exit__(None, None, None)
                    wo_pool = ctx.enter_context(
                        tc.tile_pool(name="wo_pool", bufs=1, side="right"))
                    wo_sb = wo_pool.tile([128, NQ, H], f16, name="wo_sb")
                    wo_src = wo_ap.rearrange("(a p) m -> p a m", p=128)
                    for hc in range(4):
                        nc.sync.dma_start(
                            out=wo_sb[:, :, hc * 1024:(hc + 1) * 1024],
                            in_=wo_src[:, :, hc * 1024:(hc + 1) * 1024])

                # normalize of the previous group overlaps this projection
                if pending is not None:
                    normalize(*pending)

                # ---- attention for q-group g ----
                nkc = 4 * g + 4   # causal k-chunks for this group
                lA = held_ps.tile([128, 512], f32, tag="lA")
                lB = held_ps.tile([128, 512], f32, tag="lB")
                lrow = [lA[0:1, :], lA[64:65, :], lB[0:1, :], lB[64:65, :]]
                Ops = [held_ps.tile([128, 512], f32, tag=f"O{h}",
                                    name=f"O{g}_{h}")
                       for h in range(NQ)]
                for jj in range(nkc):
                    PT = ptp.tile([128, NQ, 512], f16, tag="PT")
                    # wave 1: all scores + exp first, so the l/PV matmuls
                    # (which wait on exp) never block the in-order PE queue
                    for h in range(NQ):
                        sps = sp_ps.tile([128, 512], f32, tag="sp")
                        nc.tensor.matmul(
                            sps[:],
                            lhsT=qkT_sb[:, 4, jj * 128:(jj + 1) * 128],
                            rhs=qkT_sb[:, h, g * 512:(g + 1) * 512],
                            start=True, stop=True)
                        nc.scalar.activation(PT[:, h, :], sps[:], Exp,
                                             scale=SCALE, bias=nbias[:])
                        if jj >= 4 * g:
                            nc.vector.tensor_mul(
                                PT[:, h, :], PT[:, h, :],
                                mask_sb[:, jj - 4 * g, :])
                    # wave 2: accumulate denominators and PV
                    for h in range(NQ):
                        nc.tensor.matmul(
                            lrow[h],
                            lhsT=ones_col[:],
                            rhs=PT[:, h, :],
                            start=(jj == 0), stop=(jj == nkc - 1))
                        nc.tensor.matmul(
                            Ops[h][:],
                            lhsT=v_sb[:, jj, :],
                            rhs=PT[:, h, :],
                            start=(jj == 0), stop=(jj == nkc - 1))
                pending = (g, lrow, Ops)
                xT_cur = xT_next if g + 1 < NG else None

            normalize(*pending)

            # ---- release the held PSUM banks for the y-proj rotation ----
            held_ps_cm.__exit__(None, None, None)
            y_ps = ctx.enter_context(
                tc.tile_pool(name="y_ps", bufs=4, space="PSUM"))

            # ---- output projection y^T = wo^T @ O^T ----
            for g in range(NG):
                for ym in range(H // 128):
                    yps = y_ps.tile([128, 512], f32, tag="y")
                    for kc in range(NQ):
                        nc.tensor.matmul(
                            yps[:],
                            lhsT=wo_sb[:, kc, ym * 128:(ym + 1) * 128],
                            rhs=outT_sb[:, kc, g * 512:(g + 1) * 512],
                            start=(kc == 0), stop=(kc == NQ - 1))
                    ye = yep.tile([128, 512], f16, tag="ye")
                    if ym % 2:
                        nc.scalar.copy(out=ye[:], in_=yps[:])
                    else:
                        nc.vector.tensor_copy(out=ye[:], in_=yps[:])
                    (nc.sync if ym % 2 else nc.scalar).dma_start(
                        out=yT_ap[ym * 128:(ym + 1) * 128,
                                  g * 512:(g + 1) * 512],
                        in_=ye[:])

    nc.compile()
    return nc


def _get_nc():
    if "nc" not in _CACHE:
        _CACHE["nc"] = _build()
    return _CACHE["nc"]


def _host_consts(rope_cache):
    rc = np.asarray(rope_cache, np.float32)          # [S, 64, 2]
    cos = rc[:, :, 0].T                              # [64, S]
    sin = rc[:, :, 1].T
    cosI = np.repeat(cos, 2, axis=0).astype(np.float16)   # [128, S]
    sinI = np.repeat(sin, 2, axis=0)
    sinI[0::2] *= -1.0                               # even rows: -sin
    sinI = sinI.astype(np.float16)

    idx = np.arange(128)
    perm = np.zeros((128, 128), np.float16)
    perm[idx ^ 1, idx] = 1.0                         # out[p] = in[p^1]

    kp = np.arange(128)[:, None]
    qf = np.arange(128)[None, :]
    tri = (qf >= kp)                                 # [128, 128] visible
    mask4 = np.zeros((128, 4, 4, 128), np.float16)   # [kp, r, c, qf]
    for r in range(4):
        for c in range(4):
            if c > r:
                mask4[:, r, c, :] = 1.0
            elif c == r:
                mask4[:, r, c, :] = tri
    mask4 = mask4.reshape(128, 4 * 512)
    return cosI, sinI, perm, mask4


def kernel(x, last_pos, mask, rope_cache, wqkv, wo):
    global LAST_RESULTS
    from concourse.bass_utils import run_bass_kernel_spmd

    nc = _get_nc()

    x2 = np.asarray(x, np.float32).reshape(S, H)
    xT16 = np.ascontiguousarray(x2.T.astype(np.float16))
    wq = np.asarray(wqkv, np.float32)
    wo_f = np.asarray(wo, np.float32)
    cosI, sinI, perm, mask4 = _host_consts(rope_cache)

    in_maps = []
    for c in range(N_CORES):
        wcat = np.concatenate(
            [wq[:, c * 512:(c + 1) * 512],
             wq[:, H + c * 128:H + (c + 1) * 128],
             wq[:, H + 1024 + c * 128:H + 1024 + (c + 1) * 128]],
            axis=1).astype(np.float16)
        in_maps.append({
            "xT": xT16,
            "w": np.ascontiguousarray(wcat),
            "wo": np.ascontiguousarray(
                wo_f[c * 512:(c + 1) * 512, :].astype(np.float16)),
            "cosI": cosI,
            "sinI": sinI,
            "perm": perm,
            "mask4": mask4,
        })

    res = run_bass_kernel_spmd(nc, in_maps, list(range(N_CORES)))
    LAST_RESULTS = res
    if res.exec_time_ns is not None:
        print(f"HW exec time: {res.exec_time_ns} ns")
    yT = res.results[0]["yT"].astype(np.float32)
    for c in range(1, N_CORES):
        yT = yT + res.results[c]["yT"].astype(np.float32)
    return np.ascontiguousarray(yT.T).reshape(1, S, H).astype(np.float32)
